# revision 1
# baseline (speedup 1.0000x reference)
"""Trainium2 Bass kernel for ConstrainedAttentionModel.

Math (per batch b):
  q_i = x[T-1-i], i in [0,8)
  scores[t] = sum_{i,j} C[i,j] * (x[t-j] == q_i), t-j >= 0;  scores[T-1] = -inf
  attn = softmax(scores over t)
  out[v] = sum_t attn[t] * (x[t] == v)          # weighted histogram, V=32000

Device strategy (8 NeuronCores, data-parallel over batch, 8 batches/core):
  Stage A (scores): polyphase decomposition t = 8u+s. Equality masks
    P[(i,b2,s), u] built with one int16 tensor_scalar(is_equal) per batch-pair
    (128 partitions = 8i x 2b x 8s). Two fp16 matmuls with host-built band
    matrices W0/W1 (from C) accumulate scores into PSUM [16=(b2,r), 2048=u].
    ACT exp with accum_out gives e = exp(scores) (fp16) + per-partition row
    sums; T-1 masked by writing -30 into its PSUM cell before exp.
  Z: PE transpose + free-dim reduce + reciprocal -> 1/Z per batch, broadcast.
  Stage B (histogram): v = 256*hi + lo. Per 128-token chunk, DVE builds
    W = (iota256==lo)*e [128,256] fp16 and U = (iota128==hi) [128,128] fp16
    (fused is_equal+mult tensor_scalar); PE contracts U^T @ W into a PSUM
    accumulator [128=hi, 256=lo] over 128 chunks/batch. Final ACT mul by 1/Z
    and DMA of [125,256] -> out[b, 0:32000].

e is exactly 1.0 in fp16 for the ~99.8% of positions with score 0, so the
histogram is near-exact; only positions in the 8-wide window after a q-token
match carry fp16 rounding (~5e-4 relative).
"""

import sys

sys.path.insert(0, "/opt/trn_rl_repo")
sys.path.insert(0, "/root/.axon_site/_ro/trn_rl_repo")

import numpy as np

import concourse.bass as bass
import concourse.mybir as mybir
import concourse.tile as tile
from concourse import bacc
from concourse.bass_utils import run_bass_kernel_spmd

B, T, KW, V = 64, 16384, 8, 32000
NCORES = 8
BPC = B // NCORES        # 8 batches per core
NPAIR = BPC // 2         # 4 batch pairs
U = T // KW              # 2048 phase columns
UC = U + 1               # +1 left halo column
UCP = 2052               # padded pair block (mult of 4)
LO = 256                 # low bins per hi slab
HI = 128                 # hi one-hot width (values 0..124 used)
HIV = V // LO            # 125 valid hi rows
CHUNKS = T // 128        # 128 token chunks per batch
GP_MOD, GP_CNT = 16, 0   # chunks k with (k % GP_MOD) < GP_CNT build on GPSIMD

DT = mybir.dt
OP = mybir.AluOpType
ACTF = mybir.ActivationFunctionType

_CACHE = {}


def _build(reps=1, variant="full"):
    nc = bacc.Bacc("TRN2", target_bir_lowering=False, debug=False,
                   num_devices=NCORES)

    x_ph = nc.dram_tensor("x_ph", [16, NPAIR * UCP], DT.int16,
                          kind="ExternalInput")
    qcol = nc.dram_tensor("qcol", [128, NPAIR], DT.float32, kind="ExternalInput")
    w0 = nc.dram_tensor("w0", [128, 16], DT.float16, kind="ExternalInput")
    w1 = nc.dram_tensor("w1", [128, 16], DT.float16, kind="ExternalInput")
    iotas = nc.dram_tensor("iotas", [128, LO + HI], DT.float16,
                           kind="ExternalInput")
    lo_sc = nc.dram_tensor("lo_sc", [128, BPC * 128], DT.float32,
                           kind="ExternalInput")
    hi_sc = nc.dram_tensor("hi_sc", [128, BPC * 128], DT.float32,
                           kind="ExternalInput")
    ident = nc.dram_tensor("ident", [128, 128], DT.float32, kind="ExternalInput")
    maskc = nc.dram_tensor("maskc", [128, 1], DT.float32, kind="ExternalInput")
    out_t = nc.dram_tensor("out", [BPC, V], DT.float32, kind="ExternalOutput")

    e_hbm = nc.dram_tensor("e_hbm", [BPC, T], DT.float32)
    zr_hbm = nc.dram_tensor("zr_hbm", [16], DT.float32)

    with tile.TileContext(nc) as tc:
        with (
            tc.tile_pool(name="big", bufs=1) as big,
            tc.tile_pool(name="wb", bufs=4) as wb,
            tc.tile_pool(name="ub", bufs=4) as ub,
            tc.tile_pool(name="psA", bufs=1, space="PSUM") as psA,
            tc.tile_pool(name="psB", bufs=2, space="PSUM") as psB,
            tc.tile_pool(name="small", bufs=1) as small,
        ):
            # ---- loads ----
            xrep = big.tile([128, NPAIR * UCP], DT.int16)
            for i in range(8):
                nc.sync.dma_start(out=xrep[16 * i:16 * (i + 1), :], in_=x_ph[:, :])
            qcol_sb = small.tile([128, NPAIR], DT.float32)
            nc.sync.dma_start(out=qcol_sb[:], in_=qcol[:, :])
            w0_sb = small.tile([128, 16], DT.float16)
            nc.sync.dma_start(out=w0_sb[:], in_=w0[:, :])
            w1_sb = small.tile([128, 16], DT.float16)
            nc.sync.dma_start(out=w1_sb[:], in_=w1[:, :])
            iota_sb = small.tile([128, LO + HI], DT.float16)
            nc.sync.dma_start(out=iota_sb[:], in_=iotas[:, :])
            lo_sb = small.tile([128, BPC * 128], DT.float32)
            nc.sync.dma_start(out=lo_sb[:], in_=lo_sc[:, :])
            hi_sb = small.tile([128, BPC * 128], DT.float32)
            nc.sync.dma_start(out=hi_sb[:], in_=hi_sc[:, :])
            id_sb = small.tile([128, 128], DT.float32)
            nc.sync.dma_start(out=id_sb[:], in_=ident[:, :])
            mask_sb = small.tile([128, 1], DT.float32)
            nc.sync.dma_start(out=mask_sb[:], in_=maskc[:, :])

            # ---- compute body (repeated `reps` times for timing runs) ----
            for _rep in range(reps):
              # ---- stage A: equality phases + score matmuls ----
              P = big.tile([128, NPAIR * UCP], DT.float16)
              for p in range(NPAIR):
                  nc.vector.tensor_scalar(
                      out=P[:, p * UCP:(p + 1) * UCP],
                      in0=xrep[:, p * UCP:(p + 1) * UCP],
                      scalar1=qcol_sb[:, p:p + 1], scalar2=None,
                      op0=OP.is_equal)

              scores = psA.tile([128, U], DT.float32, space="PSUM")
              NT = U // 512
              for p in range(NPAIR):
                  for n in range(NT):
                      nc.tensor.matmul(
                          out=scores[32 * p:32 * p + 16, 512 * n:512 * (n + 1)],
                          lhsT=w0_sb[:],
                          rhs=P[:, p * UCP + 1 + 512 * n: p * UCP + 1 + 512 * (n + 1)],
                          start=True, stop=False, tile_position=(0, 32 * p))
              for p in range(NPAIR):
                  for n in range(NT):
                      nc.tensor.matmul(
                          out=scores[32 * p:32 * p + 16, 512 * n:512 * (n + 1)],
                          lhsT=w1_sb[:],
                          rhs=P[:, p * UCP + 512 * n: p * UCP + 512 * (n + 1)],
                          start=False, stop=True, tile_position=(0, 32 * p))

              # mask t = T-1: add -30 to its score cell (host mask vector)
              nc.vector.tensor_tensor(
                  out=scores[:, U - 1:U], in0=scores[:, U - 1:U],
                  in1=mask_sb[:], op=OP.add)

              e_sb = big.tile([128, U], DT.float32)
              zpart = small.tile([128, 1], DT.float32)
              nc.vector.memset(zpart[:], 0.0)
              for p in range(NPAIR):
                  nc.scalar.activation(
                      out=e_sb[32 * p:32 * p + 16, :],
                      in_=scores[32 * p:32 * p + 16, :],
                      func=ACTF.Exp,
                      accum_out=zpart[32 * p:32 * p + 16, 0:1])


              # ---- Z = sum over r; 1/Z broadcast ----
              zT = psB.tile([1, 128], DT.float32, space="PSUM")
              nc.tensor.transpose(out=zT[:], in_=zpart[:], identity=id_sb[:])
              zT_sb = small.tile([1, 128], DT.float32)
              nc.vector.tensor_copy(out=zT_sb[:], in_=zT[:])
              zsum = small.tile([1, 16], DT.float32)
              nc.vector.tensor_reduce(
                  out=zsum[0:1, :],
                  in_=zT_sb[0:1, :].rearrange("p (g r) -> p g r", r=8),
                  axis=mybir.AxisListType.X, op=OP.add)
              zrec = small.tile([1, 16], DT.float32)
              nc.vector.reciprocal(out=zrec[:], in_=zsum[:])
              nc.sync.dma_start(out=zr_hbm[:], in_=zrec[0:1, :])
              zrb = small.tile([128, 16], DT.float32)
              nc.sync.dma_start(out=zrb[:], in_=bass.AP(zr_hbm, 0, [[0, 128], [1, 16]]))

              # ---- e bounce to scatter layout ----
              e_sc = small.tile([128, BPC * 128], DT.float32)
              for b in range(BPC):
                  pb = 32 * (b // 2) + 8 * (b % 2)
                  nc.sync.dma_start(
                      out=e_hbm[b].rearrange("(u r) -> r u", r=8),
                      in_=e_sb[pb:pb + 8, :])
              for b in range(BPC):
                  nc.sync.dma_start(
                      out=e_sc[:, 128 * b:128 * (b + 1)],
                      in_=e_hbm[b].rearrange("(p f) -> p f", p=128))

              # ---- stage B: weighted histogram ----
              if variant == "stageA":
                  continue
              do_w = variant in ("full", "nomm", "wonly")
              do_u = variant in ("full", "nomm", "uonly")
              do_mm = variant == "full"
              wprev = uprev = None
              for b in range(BPC):
                  hist = psB.tile([128, LO], DT.float32, space="PSUM", tag="hist")
                  for k in range(CHUNKS):
                      col = 128 * b + k
                      eng = nc.gpsimd if (k % GP_MOD) < GP_CNT else nc.vector
                      if do_w:
                          wt = wb.tile([128, LO], DT.float16, tag="wt")
                          w_in0 = iota_sb[:, 0:LO] if (do_mm or wprev is None) \
                              else wprev[:]
                          eng.tensor_scalar(
                              out=wt[:], in0=w_in0,
                              scalar1=lo_sb[:, col:col + 1],
                              scalar2=e_sc[:, col:col + 1],
                              op0=OP.is_equal, op1=OP.mult)
                          wprev = wt
                      if do_u:
                          ut = ub.tile([128, HI], DT.float16, tag="ut")
                          u_in0 = iota_sb[:, LO:LO + HI] if (do_mm or uprev is None) \
                              else uprev[:]
                          eng.tensor_scalar(
                              out=ut[:], in0=u_in0,
                              scalar1=hi_sb[:, col:col + 1], scalar2=None,
                              op0=OP.is_equal)
                          uprev = ut
                      if do_mm:
                          nc.tensor.matmul(out=hist[:], lhsT=ut[:], rhs=wt[:],
                                           start=(k == 0), stop=(k == CHUNKS - 1))
                  if not do_mm:
                      continue
                  hist_sb = wb.tile([128, LO], DT.float32, tag="hsb")
                  g = 4 * (b // 2) + (b % 2)
                  nc.scalar.mul(out=hist_sb[:], in_=hist[:], mul=zrb[:, g:g + 1])
                  nc.sync.dma_start(
                      out=out_t[b].rearrange("(h l) -> h l", h=HIV),
                      in_=hist_sb[0:HIV, :])
              if not do_mm:
                  # keep chained builds alive past DCE
                  keep = wprev if wprev is not None else uprev
                  nc.gpsimd.dma_start(out=e_hbm[0, 0:keep.shape[1]].rearrange(
                      "(p f) -> p f", p=1), in_=keep[0:1, :])

    nc.compile()
    return nc


def _host_prep(xs):
    """Per-core input arrays from xs int32 [BPC, T] and shared consts."""
    xpad = np.full((BPC, 8 + T), -1, np.int16)
    xpad[:, 8:] = xs.astype(np.int16)
    view = xpad.reshape(BPC, UC, 8)              # [b, c, s]
    x_ph = np.full((16, NPAIR * UCP), -3, np.int16)
    for pair in range(NPAIR):
        for b2 in range(2):
            # rows 8*b2+s, cols pair*UCP + c
            x_ph[8 * b2:8 * (b2 + 1), pair * UCP:pair * UCP + UC] = \
                view[2 * pair + b2].T
    q = xs[:, T - 1 - np.arange(KW)]             # [BPC, 8] int32
    qcol = np.zeros((128, NPAIR), np.float32)
    for i in range(KW):
        for b2 in range(2):
            for pair in range(NPAIR):
                qcol[16 * i + 8 * b2:16 * i + 8 * b2 + 8, pair] = q[2 * pair + b2, i]
    arr = xs.reshape(BPC, 128, 128)              # [b, p, k], t = 128p + k
    lo_sc = np.ascontiguousarray(
        (arr & 255).transpose(1, 0, 2).reshape(128, BPC * 128)).astype(np.float32)
    hi_sc = np.ascontiguousarray(
        (arr >> 8).transpose(1, 0, 2).reshape(128, BPC * 128)).astype(np.float32)
    return x_ph, qcol, lo_sc, hi_sc


def _shared_consts(C):
    w0 = np.zeros((128, 16), np.float16)
    w1 = np.zeros((128, 16), np.float16)
    Ch = C.astype(np.float16)
    for i in range(KW):
        for b2 in range(2):
            for s in range(KW):
                row = 16 * i + 8 * b2 + s
                for r in range(KW):
                    m = 8 * b2 + r
                    if r >= s:
                        w0[row, m] = Ch[i, r - s]
                    else:
                        w1[row, m] = Ch[i, r - s + 8]
    iotas = np.zeros((128, LO + HI), np.float16)
    iotas[:, :LO] = np.arange(LO, dtype=np.float16)[None, :]
    iotas[:, LO:] = np.arange(HI, dtype=np.float16)[None, :]
    ident = np.eye(128, dtype=np.float32)
    maskc = np.zeros((128, 1), np.float32)
    for b in range(BPC):
        maskc[32 * (b // 2) + 8 * (b % 2) + 7, 0] = -30.0
    return w0, w1, iotas, ident, maskc


def _get_runner(reps=1, variant="full"):
    """Cached sharded PJRT callable (bass2jax re-traces per call otherwise)."""
    key = ("runner", reps, variant)
    if key in _CACHE:
        return _CACHE[key]
    nc = _build(reps, variant)

    import jax
    from jax.experimental.shard_map import shard_map
    from jax.sharding import Mesh, PartitionSpec
    import concourse.mybir as mb
    from concourse import bass2jax

    bass2jax.install_neuronx_cc_hook()
    pname = nc.partition_id_tensor.name if nc.partition_id_tensor else None
    in_names, out_names, out_avals = [], [], []
    for alloc in nc.m.functions[0].allocations:
        if not isinstance(alloc, mb.MemoryLocationSet):
            continue
        name = alloc.memorylocations[0].name
        if alloc.kind == "ExternalInput":
            if name == pname:
                continue
            in_names.append(name)
        elif alloc.kind == "ExternalOutput":
            out_names.append(name)
            out_avals.append(jax.core.ShapedArray(
                tuple(alloc.tensor_shape), mb.dt.np(alloc.dtype)))
    n_params = len(in_names)
    all_names = tuple(in_names + out_names + ([pname] if pname else []))
    n_outs = len(out_names)

    def _body(*args):
        operands = list(args)
        if pname is not None:
            operands.append(bass2jax.partition_id_tensor())
        outs = bass2jax._bass_exec_p.bind(
            *operands, out_avals=tuple(out_avals), in_names=all_names,
            out_names=tuple(out_names), lowering_input_output_aliases=(),
            sim_require_finite=True, sim_require_nnan=True, nc=nc)
        return tuple(outs)

    devices = jax.devices()[:NCORES]
    mesh = Mesh(np.asarray(devices), ("core",))
    in_specs = (PartitionSpec("core"),) * (n_params + n_outs)
    out_specs = (PartitionSpec("core"),) * n_outs
    sharded = jax.jit(
        shard_map(_body, mesh=mesh, in_specs=in_specs, out_specs=out_specs,
                  check_rep=False),
        keep_unused=True)

    runner = dict(fn=sharded, in_names=in_names, out_names=out_names,
                  out_avals=out_avals)
    _CACHE[key] = runner
    return runner


def _make_concat_inputs(C, x, reps=1, variant="full"):
    w0, w1, iotas, ident, maskc = _shared_consts(C)
    xi = np.asarray(x).astype(np.int32)
    in_maps = []
    for c in range(NCORES):
        x_ph, qcol, lo_sc, hi_sc = _host_prep(xi[BPC * c:BPC * (c + 1)])
        in_maps.append(dict(x_ph=x_ph, qcol=qcol, w0=w0, w1=w1, iotas=iotas,
                            lo_sc=lo_sc, hi_sc=hi_sc, ident=ident, maskc=maskc))
    r = _get_runner(reps, variant)
    concat = [np.concatenate([m[n] for m in in_maps], axis=0)
              for n in r["in_names"]]
    zeros = [np.zeros((NCORES * a.shape[0], *a.shape[1:]), a.dtype)
             for a in r["out_avals"]]
    return concat, zeros


def _run(concat, zeros, reps=1, variant="full"):
    r = _get_runner(reps, variant)
    out_arrs = r["fn"](*concat, *zeros)
    i = r["out_names"].index("out")
    return np.asarray(out_arrs[i]).reshape(NCORES * BPC, V)


def kernel(C, x, vocab_size):
    C = np.asarray(C, np.float32)
    x = np.asarray(x)
    assert x.shape == (B, T) and int(vocab_size) == V
    concat, zeros = _make_concat_inputs(C, x)
    return _run(concat, zeros).astype(np.float32)



# revision 4
# speedup vs baseline: 2.5932x; 2.5932x over previous
"""Trainium2 Bass kernel for ConstrainedAttentionModel.

Math (per batch b):
  q_i = x[T-1-i], i in [0,8)
  scores[t] = sum_{i,j} C[i,j] * (x[t-j] == q_i), t-j >= 0;  scores[T-1] = -inf
  attn = softmax(scores over t)
  out[v] = sum_t attn[t] * (x[t] == v)          # weighted histogram, V=32000

Device strategy (8 NeuronCores, data-parallel over batch, 8 batches/core):
  Host uploads ONLY x (int16 [BPC, T], 262KB/core) plus ~20KB of consts.
  All layouts are derived on device:
    - polyphase tile x_ph [16=(b2,s), pair*UCP] via strided DMA from x,
      replicated 8x along partitions (i index) through an HBM bounce;
    - scatter tile x_sc [128=p, BPC*128] (t = 128p+k) via direct DMA;
    - lo = x & 255, hi = x >> 8 (DVE int ops); iota rows via GPSIMD iota.
  Stage A (scores): equality phases P[(i,b2,s), u] = (x_tok == q_i) via one
    tensor_scalar(is_equal) per batch pair; two fp16 matmuls with host-built
    band matrices W0/W1 (from C) accumulate scores into PSUM [16=(b2,r), 2048=u].
    ACT exp with accum_out gives e = exp(scores) + per-partition row sums;
    t=T-1 masked by adding -30 to its PSUM cell.
  Z: one matmul zpart[128,1]^T @ gmask[128,16] -> [1,16] batch sums; then
    reciprocal * 2^14; broadcast to [128,16] via HBM bounce.
  Stage B (histogram): v = 256*hi + lo. Per 128-token chunk, DVE builds
    W = (iota256==lo)*e [128,256] fp16 and U = (iota128==hi) [128,128] fp16;
    PE contracts U^T @ W into PSUM [128=hi, 256=lo] over 128 chunks/batch.
    Final ACT mul by 2^14/Z -> fp16, DMA [125,256] -> out[b, 0:32000].
  Output is fp16 scaled by 2^14 (values ~= count, well inside fp16 range);
  host multiplies by 2^-14 during the f32 conversion (exact power of two).

e is exactly 1.0 in fp16 for the ~99.8% of positions with score 0, so the
histogram is near-exact; only positions in the 8-wide window after a q-token
match carry fp16 rounding (~5e-4 relative).
"""

import sys

sys.path.insert(0, "/opt/trn_rl_repo")
sys.path.insert(0, "/root/.axon_site/_ro/trn_rl_repo")

import numpy as np

import concourse.bass as bass
import concourse.mybir as mybir
import concourse.tile as tile
from concourse import bacc
from concourse.bass_utils import run_bass_kernel_spmd  # noqa: F401 (env contract)

B, T, KW, V = 64, 16384, 8, 32000
NCORES = 8
BPC = B // NCORES        # 8 batches per core
NPAIR = BPC // 2         # 4 batch pairs
U = T // KW              # 2048 phase columns
UC = U + 1               # +1 left halo column
UCP = 2052               # padded pair block (mult of 4)
LO = 256                 # low bins per hi slab
HI = 128                 # hi one-hot width (values 0..124 used)
HIV = V // LO            # 125 valid hi rows
CHUNKS = T // 128        # 128 token chunks per batch
OUT_SCALE = 2.0 ** 14    # device multiplies by 2^14/Z; host by 2^-14

DT = mybir.dt
OP = mybir.AluOpType
ACTF = mybir.ActivationFunctionType

_CACHE = {}


def _build(reps=1, variant="full"):
    nc = bacc.Bacc("TRN2", target_bir_lowering=False, debug=False,
                   num_devices=NCORES)

    x_t = nc.dram_tensor("x", [BPC, T], DT.int16, kind="ExternalInput")
    qcol = nc.dram_tensor("qcol", [128, NPAIR], DT.float32, kind="ExternalInput")
    w0 = nc.dram_tensor("w0", [128, 16], DT.float16, kind="ExternalInput")
    w1 = nc.dram_tensor("w1", [128, 16], DT.float16, kind="ExternalInput")
    maskc = nc.dram_tensor("maskc", [128, 1], DT.float32, kind="ExternalInput")
    gmask = nc.dram_tensor("gmask", [128, 16], DT.float32, kind="ExternalInput")
    out_t = nc.dram_tensor("out", [BPC, V], DT.float16, kind="ExternalOutput")

    e_hbm = nc.dram_tensor("e_hbm", [BPC, T], DT.float32)
    zr_hbm = nc.dram_tensor("zr_hbm", [16], DT.float32)
    xph_hbm = nc.dram_tensor("xph_hbm", [16, NPAIR * UCP], DT.int16)

    with tile.TileContext(nc) as tc:
        with (
            tc.tile_pool(name="big", bufs=1) as big,
            tc.tile_pool(name="wb", bufs=4) as wb,
            tc.tile_pool(name="ub", bufs=4) as ub,
            tc.tile_pool(name="psA", bufs=1, space="PSUM") as psA,
            tc.tile_pool(name="psB", bufs=2, space="PSUM") as psB,
            tc.tile_pool(name="small", bufs=1) as small,
        ):
            # ---- small loads ----
            qcol_sb = small.tile([128, NPAIR], DT.float32)
            nc.sync.dma_start(out=qcol_sb[:], in_=qcol[:, :])
            w0_sb = small.tile([128, 16], DT.float16)
            nc.sync.dma_start(out=w0_sb[:], in_=w0[:, :])
            w1_sb = small.tile([128, 16], DT.float16)
            nc.sync.dma_start(out=w1_sb[:], in_=w1[:, :])
            mask_sb = small.tile([128, 1], DT.float32)
            nc.sync.dma_start(out=mask_sb[:], in_=maskc[:, :])
            gmask_sb = small.tile([128, 16], DT.float32)
            nc.sync.dma_start(out=gmask_sb[:], in_=gmask[:, :])

            # ---- iota rows (device-generated consts) ----
            iota_i16 = small.tile([128, LO + HI], DT.int16)
            nc.gpsimd.iota(out=iota_i16[:, 0:LO], pattern=[[1, LO]],
                           base=0, channel_multiplier=0)
            nc.gpsimd.iota(out=iota_i16[:, LO:LO + HI], pattern=[[1, HI]],
                           base=0, channel_multiplier=0)
            iota_sb = small.tile([128, LO + HI], DT.float16)
            nc.vector.tensor_copy(out=iota_sb[:], in_=iota_i16[:])

            # ---- polyphase layout: x -> x_ph rows (b2,s), cols (pair, u+1) ----
            x_ph_sb = small.tile([16, NPAIR * UCP], DT.int16)
            nc.vector.memset(x_ph_sb[:], -1)
            for pair in range(NPAIR):
                for b2 in range(2):
                    nc.sync.dma_start(
                        out=x_ph_sb[8 * b2:8 * b2 + 8,
                                    pair * UCP + 1:pair * UCP + 1 + U],
                        in_=x_t[2 * pair + b2].rearrange("(u s) -> s u", s=KW))
            nc.sync.dma_start(out=xph_hbm[:, :], in_=x_ph_sb[:])
            xrep = big.tile([128, NPAIR * UCP], DT.int16)
            for i in range(8):
                nc.sync.dma_start(out=xrep[16 * i:16 * (i + 1), :],
                                  in_=xph_hbm[:, :])

            # ---- scatter layout + lo/hi decomposition ----
            x_sc = small.tile([128, BPC * 128], DT.int16)
            for b in range(BPC):
                nc.sync.dma_start(
                    out=x_sc[:, 128 * b:128 * (b + 1)],
                    in_=x_t[b].rearrange("(p f) -> p f", p=128))
            lo_i = small.tile([128, BPC * 128], DT.int16)
            nc.vector.tensor_scalar(out=lo_i[:], in0=x_sc[:], scalar1=255,
                                    scalar2=None, op0=OP.bitwise_and)
            hi_i = small.tile([128, BPC * 128], DT.int16)
            nc.vector.tensor_scalar(out=hi_i[:], in0=x_sc[:], scalar1=8,
                                    scalar2=None, op0=OP.logical_shift_right)
            lo_sb = small.tile([128, BPC * 128], DT.float32)
            nc.vector.tensor_copy(out=lo_sb[:], in_=lo_i[:])
            hi_sb = small.tile([128, BPC * 128], DT.float32)
            nc.vector.tensor_copy(out=hi_sb[:], in_=hi_i[:])

            # ---- compute body (repeated `reps` times for timing runs) ----
            for _rep in range(reps):
              # ---- stage A: equality phases + score matmuls ----
              P = big.tile([128, NPAIR * UCP], DT.float16)
              for p in range(NPAIR):
                  nc.vector.tensor_scalar(
                      out=P[:, p * UCP:(p + 1) * UCP],
                      in0=xrep[:, p * UCP:(p + 1) * UCP],
                      scalar1=qcol_sb[:, p:p + 1], scalar2=None,
                      op0=OP.is_equal)

              scores = psA.tile([128, U], DT.float32, space="PSUM")
              NT = U // 512
              for p in range(NPAIR):
                  for n in range(NT):
                      nc.tensor.matmul(
                          out=scores[32 * p:32 * p + 16, 512 * n:512 * (n + 1)],
                          lhsT=w0_sb[:],
                          rhs=P[:, p * UCP + 1 + 512 * n: p * UCP + 1 + 512 * (n + 1)],
                          start=True, stop=False, tile_position=(0, 32 * p))
              for p in range(NPAIR):
                  for n in range(NT):
                      nc.tensor.matmul(
                          out=scores[32 * p:32 * p + 16, 512 * n:512 * (n + 1)],
                          lhsT=w1_sb[:],
                          rhs=P[:, p * UCP + 512 * n: p * UCP + 512 * (n + 1)],
                          start=False, stop=True, tile_position=(0, 32 * p))

              # mask t = T-1: add -30 to its score cell (host mask vector)
              nc.vector.tensor_tensor(
                  out=scores[:, U - 1:U], in0=scores[:, U - 1:U],
                  in1=mask_sb[:], op=OP.add)

              e_sb = big.tile([128, U], DT.float32)
              zpart = small.tile([128, 1], DT.float32)
              nc.vector.memset(zpart[:], 0.0)
              for p in range(NPAIR):
                  nc.scalar.activation(
                      out=e_sb[32 * p:32 * p + 16, :],
                      in_=scores[32 * p:32 * p + 16, :],
                      func=ACTF.Exp,
                      accum_out=zpart[32 * p:32 * p + 16, 0:1])

              # ---- Z = per-batch sum via masked matmul; 2^14/Z broadcast ----
              zsum_ps = psB.tile([1, 16], DT.float32, space="PSUM", tag="zs")
              nc.tensor.matmul(out=zsum_ps[:], lhsT=zpart[:, 0:1],
                               rhs=gmask_sb[:], start=True, stop=True)
              zsum_sb = small.tile([1, 16], DT.float32)
              nc.vector.tensor_copy(out=zsum_sb[:], in_=zsum_ps[:])
              zrec = small.tile([1, 16], DT.float32)
              nc.vector.reciprocal(out=zrec[:], in_=zsum_sb[:])
              zrec2 = small.tile([1, 16], DT.float32)
              nc.vector.tensor_scalar(out=zrec2[:], in0=zrec[:],
                                      scalar1=float(OUT_SCALE), scalar2=None,
                                      op0=OP.mult)
              nc.sync.dma_start(out=zr_hbm[:], in_=zrec2[0:1, :])
              zrb = small.tile([128, 16], DT.float32)
              nc.sync.dma_start(out=zrb[:], in_=bass.AP(zr_hbm, 0, [[0, 128], [1, 16]]))

              # ---- e bounce to scatter layout ----
              e_sc = small.tile([128, BPC * 128], DT.float32)
              for b in range(BPC):
                  pb = 32 * (b // 2) + 8 * (b % 2)
                  nc.sync.dma_start(
                      out=e_hbm[b].rearrange("(u r) -> r u", r=8),
                      in_=e_sb[pb:pb + 8, :])
              for b in range(BPC):
                  nc.sync.dma_start(
                      out=e_sc[:, 128 * b:128 * (b + 1)],
                      in_=e_hbm[b].rearrange("(p f) -> p f", p=128))

              # ---- stage B: weighted histogram ----
              if variant == "stageA":
                  continue
              for b in range(BPC):
                  hist = psB.tile([128, LO], DT.float32, space="PSUM", tag="hist")
                  for k in range(CHUNKS):
                      col = 128 * b + k
                      wt = wb.tile([128, LO], DT.float16, tag="wt")
                      nc.vector.tensor_scalar(
                          out=wt[:], in0=iota_sb[:, 0:LO],
                          scalar1=lo_sb[:, col:col + 1],
                          scalar2=e_sc[:, col:col + 1],
                          op0=OP.is_equal, op1=OP.mult)
                      ut = ub.tile([128, HI], DT.float16, tag="ut")
                      nc.vector.tensor_scalar(
                          out=ut[:], in0=iota_sb[:, LO:LO + HI],
                          scalar1=hi_sb[:, col:col + 1], scalar2=None,
                          op0=OP.is_equal)
                      nc.tensor.matmul(out=hist[:], lhsT=ut[:], rhs=wt[:],
                                       start=(k == 0), stop=(k == CHUNKS - 1))
                  hist_sb = wb.tile([128, LO], DT.float16, tag="hsb")
                  g = 4 * (b // 2) + (b % 2)
                  nc.scalar.mul(out=hist_sb[:], in_=hist[:], mul=zrb[:, g:g + 1])
                  nc.sync.dma_start(
                      out=out_t[b].rearrange("(h l) -> h l", h=HIV),
                      in_=hist_sb[0:HIV, :])

    nc.compile()
    return nc


def _host_prep(xs):
    """Per-core input arrays from xs int32 [BPC, T]."""
    x16 = np.ascontiguousarray(xs.astype(np.int16))
    q = xs[:, T - 1 - np.arange(KW)]             # [BPC, 8] int32
    qcol = np.zeros((128, NPAIR), np.float32)
    for i in range(KW):
        for b2 in range(2):
            for pair in range(NPAIR):
                qcol[16 * i + 8 * b2:16 * i + 8 * b2 + 8, pair] = q[2 * pair + b2, i]
    return x16, qcol


def _shared_consts(C):
    w0 = np.zeros((128, 16), np.float16)
    w1 = np.zeros((128, 16), np.float16)
    Ch = C.astype(np.float16)
    for i in range(KW):
        for b2 in range(2):
            for s in range(KW):
                row = 16 * i + 8 * b2 + s
                for r in range(KW):
                    m = 8 * b2 + r
                    if r >= s:
                        w0[row, m] = Ch[i, r - s]
                    else:
                        w1[row, m] = Ch[i, r - s + 8]
    maskc = np.zeros((128, 1), np.float32)
    for b in range(BPC):
        maskc[32 * (b // 2) + 8 * (b % 2) + 7, 0] = -30.0
    gmask = np.zeros((128, 16), np.float32)
    for p in range(128):
        gmask[p, p // 8] = 1.0
    return w0, w1, maskc, gmask


def _get_runner(reps=1, variant="full"):
    """Cached sharded PJRT callable (bass2jax re-traces per call otherwise)."""
    key = ("runner", reps, variant)
    if key in _CACHE:
        return _CACHE[key]
    nc = _build(reps, variant)

    import jax
    import jax.numpy as jnp
    from jax.experimental.shard_map import shard_map
    from jax.sharding import Mesh, PartitionSpec
    import concourse.mybir as mb
    from concourse import bass2jax

    bass2jax.install_neuronx_cc_hook()
    pname = nc.partition_id_tensor.name if nc.partition_id_tensor else None
    in_names, out_names, out_avals = [], [], []
    for alloc in nc.m.functions[0].allocations:
        if not isinstance(alloc, mb.MemoryLocationSet):
            continue
        name = alloc.memorylocations[0].name
        if alloc.kind == "ExternalInput":
            if name == pname:
                continue
            in_names.append(name)
        elif alloc.kind == "ExternalOutput":
            out_names.append(name)
            out_avals.append(jax.core.ShapedArray(
                tuple(alloc.tensor_shape), mb.dt.np(alloc.dtype)))
    n_params = len(in_names)
    all_names = tuple(in_names + out_names + ([pname] if pname else []))
    n_outs = len(out_names)

    def _body(*args):
        operands = list(args)
        if pname is not None:
            operands.append(bass2jax.partition_id_tensor())
        outs = bass2jax._bass_exec_p.bind(
            *operands, out_avals=tuple(out_avals), in_names=all_names,
            out_names=tuple(out_names), lowering_input_output_aliases=(),
            sim_require_finite=True, sim_require_nnan=True, nc=nc)
        return tuple(outs)

    devices = jax.devices()[:NCORES]
    mesh = Mesh(np.asarray(devices), ("core",))
    in_specs = (PartitionSpec("core"),) * (n_params + n_outs)
    out_specs = (PartitionSpec("core"),) * n_outs
    sharded = jax.jit(
        shard_map(_body, mesh=mesh, in_specs=in_specs, out_specs=out_specs,
                  check_rep=False),
        keep_unused=True)

    # Device-resident output buffers, transferred once and reused every call
    # (bass_exec does not donate/alias its operands).
    from jax.sharding import NamedSharding
    zsh = NamedSharding(mesh, PartitionSpec("core"))
    zeros_dev = tuple(
        jax.device_put(
            np.zeros((NCORES * a.shape[0], *a.shape[1:]), a.dtype), zsh)
        for a in out_avals)
    jax.block_until_ready(zeros_dev)

    runner = dict(fn=sharded, in_names=in_names, out_names=out_names,
                  out_avals=out_avals, zeros=zeros_dev)
    _CACHE[key] = runner
    return runner


def _make_concat_inputs(C, x, reps=1, variant="full"):
    w0, w1, maskc, gmask = _shared_consts(C)
    xi = np.asarray(x).astype(np.int32)
    in_maps = []
    for c in range(NCORES):
        x16, qcol = _host_prep(xi[BPC * c:BPC * (c + 1)])
        in_maps.append(dict(x=x16, qcol=qcol, w0=w0, w1=w1, maskc=maskc,
                            gmask=gmask))
    r = _get_runner(reps, variant)
    concat = [np.concatenate([m[n] for m in in_maps], axis=0)
              for n in r["in_names"]]
    return concat


def _run(concat, reps=1, variant="full"):
    r = _get_runner(reps, variant)
    out_arrs = r["fn"](*concat, *r["zeros"])
    i = r["out_names"].index("out")
    out16 = np.asarray(out_arrs[i])
    return (out16.astype(np.float32) * np.float32(1.0 / OUT_SCALE)).reshape(
        NCORES * BPC, V)


def kernel(C, x, vocab_size):
    C = np.asarray(C, np.float32)
    x = np.asarray(x)
    assert x.shape == (B, T) and int(vocab_size) == V
    concat = _make_concat_inputs(C, x)
    return _run(concat)
